# revision 1
# baseline (speedup 1.0000x reference)
"""Deformable-conv im2col kernel for Trainium2 (8 NeuronCores, batch-parallel).

Contract: kernel(**inputs) takes the FULL inputs (data_im [8,64,128,128],
offset [8,18,128,128], mask [8,9,128,128]) and returns col [576,8,128,128].
Each of the 8 cores processes one batch image (SPMD, no collectives).

Per-core algorithm:
  1. Transpose the image NCHW -> NHWC into a DRAM scratch (PE transposes).
  2. Compute per-(tap k, output pixel) bilinear slot-weights and gather
     record indices on [wo-partition, ho-free] tiles (DVE/ACT).
  3. Rearrange indices into dma_gather's wrapped int16 layout with 8
     constant permutation matmuls on PE.
  4. Per (k, 16-row block): two dma_gathers fetch 512B row-segments
     (both x-corners x 64 channels, 256B-granular overlapping records),
     one broadcast-multiply per gather applies the 4 slot weights, and
     4 PSUM-accumulated PE transposes sum the corners while transposing
     pixels-on-partitions -> channels-on-partitions for the output DMA.
"""

import sys

sys.path.insert(0, "/opt/trn_rl_repo")

import numpy as np

import concourse.bass as bass
import concourse.bacc as bacc
import concourse.mybir as mybir
import concourse.tile as tile
from concourse.masks import make_identity
from concourse.bass_utils import run_bass_kernel_spmd

dt = mybir.dt
Alu = mybir.AluOpType
ACT = mybir.ActivationFunctionType

H = W = 128
C = 64
K = 9
HW = H * W  # 16384
J = 16      # output rows per gather round
NBLK = H // J  # 8
NREC = HW + 1  # gather records incl. one pad record


def _build():
    nc = bacc.Bacc("TRN2", target_bir_lowering=False, debug=False,
                   dynamic_dma_scratch_size=32768, num_swdge_queues=2)

    x_im = nc.dram_tensor("x_im", [C, HW], dt.float32, kind="ExternalInput")
    x_off = nc.dram_tensor("x_off", [18, HW], dt.float32, kind="ExternalInput")
    x_mask = nc.dram_tensor("x_mask", [K, HW], dt.float32, kind="ExternalInput")
    col = nc.dram_tensor("col", [C * K, HW], dt.float32, kind="ExternalOutput")
    nhwc = nc.dram_tensor("nhwc", [NREC * C], dt.float32, kind="Internal")

    with tile.TileContext(nc) as tc:
        # ---- persistent pools -------------------------------------------
        with (
            tc.tile_pool(name="const", bufs=1) as cpool,
            tc.tile_pool(name="wts", bufs=1) as wpool,
        ):
            ident = cpool.tile([128, 128], dt.float32)
            make_identity(nc, ident[:])

            psAB_cm = tc.tile_pool(name="psAB", bufs=4, space="PSUM")
            psA = psB = psAB_cm.__enter__()
            psC = psA

            # ---- phase 0: NCHW -> NHWC in DRAM --------------------------
            with tc.tile_pool(name="ph0", bufs=1) as p0, tc.tile_pool(
                name="ph0s", bufs=4
            ) as p0s:
                im_sb = p0.tile([C, HW], dt.float32)
                nc.sync.dma_start(out=im_sb[:], in_=x_im[:])
                zrow = p0s.tile([1, C], dt.float32)
                nc.gpsimd.memset(zrow[:], 0.0)
                nc.sync.dma_start(
                    out=bass.AP(nhwc, HW * C, [[1, C]]), in_=zrow[:]
                )
                for b in range(H):
                    ps = psA.tile([128, C], dt.float32, space="PSUM")
                    nc.tensor.transpose(
                        ps[:], im_sb[:, b * 128 : (b + 1) * 128], ident[0:C, 0:C]
                    )
                    st = p0s.tile([128, C], dt.float32)
                    nc.scalar.activation(st[:], ps[:], ACT.Copy)
                    nc.sync.dma_start(
                        out=bass.AP(nhwc, b * 128 * C, [[C, 128], [1, C]]),
                        in_=st[:],
                    )

            # ---- phase 1: transpose offsets+mask to [wo, q*128+ho] ------
            OT = wpool.tile([128, 27 * 128], dt.float32)
            with tc.tile_pool(name="ph1", bufs=1) as p1:
                om = p1.tile([27, HW], dt.float32)
                nc.sync.dma_start(out=om[0:18, :], in_=x_off[:])
                nc.sync.dma_start(out=om[18:27, :], in_=x_mask[:])
                for b in range(H):
                    ps = psA.tile([128, 27], dt.float32, space="PSUM")
                    nc.tensor.transpose(
                        ps[:], om[:, b * 128 : (b + 1) * 128], ident[0:27, 0:27]
                    )
                    ot = OT[:]
                    nc.scalar.activation(
                        bass.AP(ot.tensor, ot.offset + b, [ot.ap[0], [128, 27]]),
                        ps[:],
                        ACT.Copy,
                    )

            # ---- phase 2: constants -------------------------------------
            iota_i = cpool.tile([128, 128], dt.int32)
            nc.gpsimd.iota(iota_i[:], pattern=[[1, 128]], base=0, channel_multiplier=0)
            iota_ho = cpool.tile([128, 128], dt.float32)
            nc.vector.tensor_copy(iota_ho[:], iota_i[:])
            iwo_i = cpool.tile([128, 1], dt.int32)
            nc.gpsimd.iota(iwo_i[:], pattern=[[1, 1]], base=0, channel_multiplier=1)
            iwo_f = cpool.tile([128, 1], dt.float32)
            nc.vector.tensor_copy(iwo_f[:], iwo_i[:])
            kxb = cpool.tile([128, K], dt.float32)
            for k in range(K):
                kx = k % 3
                nc.vector.tensor_scalar(
                    kxb[:, k : k + 1], iwo_f[:], float(kx - 1 + 63.5), None, op0=Alu.add
                )
            ones = cpool.tile([128, 128], dt.float32)
            nc.gpsimd.memset(ones[:], 1.0)
            # R matrices: R[wg][q, p] = 1 iff q == wg*16 + p%16
            Rm = []
            for wg in range(8):
                r = cpool.tile([128, 128], dt.float32, tag=f"R{wg}")
                nc.gpsimd.affine_select(
                    r[:],
                    ones[:],
                    pattern=[[0, 8], [1, 16]],
                    base=wg * 16,
                    channel_multiplier=-1,
                    compare_op=Alu.is_equal,
                    fill=0.0,
                )
                Rm.append(r)

            # ---- phase 3: weights + wrapped indices per tap -------------
            W4s, IWts, IWbs = [], [], []
            with tc.tile_pool(name="wk", bufs=2) as wk:
                for k in range(K):
                    ky = k // 3
                    OY = OT[:, (2 * k) * 128 : (2 * k + 1) * 128]
                    OX = OT[:, (2 * k + 1) * 128 : (2 * k + 2) * 128]
                    M = OT[:, (18 + k) * 128 : (19 + k) * 128]

                    pyA = wk.tile([128, 128], dt.float32, tag="pyA")
                    nc.vector.scalar_tensor_tensor(
                        pyA[:], OY, float(ky - 1 + 63.5), iota_ho[:],
                        op0=Alu.add, op1=Alu.add,
                    )
                    pxA = wk.tile([128, 128], dt.float32, tag="pxA")
                    nc.vector.tensor_scalar(pxA[:], OX, kxb[:, k : k + 1], None, op0=Alu.add)

                    Yi = wk.tile([128, 128], dt.int32, tag="Yi")
                    nc.vector.tensor_copy(Yi[:], pyA[:])
                    Xi = wk.tile([128, 128], dt.int32, tag="Xi")
                    nc.vector.tensor_copy(Xi[:], pxA[:])
                    Yf = wk.tile([128, 128], dt.float32, tag="Yf")
                    nc.vector.tensor_copy(Yf[:], Yi[:])
                    Xf = wk.tile([128, 128], dt.float32, tag="Xf")
                    nc.vector.tensor_copy(Xf[:], Xi[:])

                    ly = wk.tile([128, 128], dt.float32, tag="ly")
                    nc.vector.scalar_tensor_tensor(
                        ly[:], pyA[:], 0.5, Yf[:], op0=Alu.add, op1=Alu.subtract
                    )
                    lx = wk.tile([128, 128], dt.float32, tag="lx")
                    nc.vector.scalar_tensor_tensor(
                        lx[:], pxA[:], 0.5, Xf[:], op0=Alu.add, op1=Alu.subtract
                    )
                    omly = wk.tile([128, 128], dt.float32, tag="omly")
                    nc.scalar.activation(omly[:], ly[:], ACT.Copy, bias=1.0, scale=-1.0)
                    omlx = wk.tile([128, 128], dt.float32, tag="omlx")
                    nc.scalar.activation(omlx[:], lx[:], ACT.Copy, bias=1.0, scale=-1.0)

                    def rng_mask(src, lo, hi, tag):
                        a = wk.tile([128, 128], dt.float32, tag=tag + "a")
                        nc.vector.tensor_scalar(a[:], src[:], lo, None, op0=Alu.is_ge)
                        b2 = wk.tile([128, 128], dt.float32, tag=tag + "b")
                        nc.vector.tensor_scalar(b2[:], src[:], hi, None, op0=Alu.is_le)
                        o = wk.tile([128, 128], dt.float32, tag=tag + "o")
                        nc.gpsimd.tensor_tensor(o[:], a[:], b2[:], op=Alu.mult)
                        return o

                    vy0 = rng_mask(Yi, 64, 191, "vy0")
                    vy1 = rng_mask(Yi, 63, 190, "vy1")
                    vx0 = rng_mask(Xi, 64, 191, "vx0")
                    vx1 = rng_mask(Xi, 63, 190, "vx1")
                    e = wk.tile([128, 128], dt.float32, tag="e")
                    nc.vector.tensor_scalar(e[:], Xi[:], 63, None, op0=Alu.is_equal)
                    ge0 = wk.tile([128, 128], dt.float32, tag="ge0")
                    nc.vector.tensor_scalar(ge0[:], Xi[:], 64, None, op0=Alu.is_ge)

                    A0 = wk.tile([128, 128], dt.float32, tag="A0")
                    nc.vector.tensor_tensor(A0[:], omly[:], vy0[:], op=Alu.mult)
                    nc.vector.tensor_tensor(A0[:], A0[:], M, op=Alu.mult)
                    A1 = wk.tile([128, 128], dt.float32, tag="A1")
                    nc.vector.tensor_tensor(A1[:], ly[:], vy1[:], op=Alu.mult)
                    nc.vector.tensor_tensor(A1[:], A1[:], M, op=Alu.mult)
                    B0 = wk.tile([128, 128], dt.float32, tag="B0")
                    nc.vector.tensor_tensor(B0[:], omlx[:], vx0[:], op=Alu.mult)
                    B1 = wk.tile([128, 128], dt.float32, tag="B1")
                    nc.vector.tensor_tensor(B1[:], lx[:], vx1[:], op=Alu.mult)
                    BB0 = wk.tile([128, 128], dt.float32, tag="BB0")
                    nc.vector.tensor_tensor(BB0[:], e[:], B1[:], op=Alu.mult)
                    nc.vector.tensor_tensor(BB0[:], BB0[:], B0[:], op=Alu.add)
                    BB1 = wk.tile([128, 128], dt.float32, tag="BB1")
                    nc.vector.tensor_tensor(BB1[:], B1[:], ge0[:], op=Alu.mult)

                    W4 = wpool.tile([128, 512], dt.float32, tag=f"W4_{k}")
                    nc.gpsimd.tensor_tensor(W4[:, 0:128], A0[:], BB0[:], op=Alu.mult)
                    nc.gpsimd.tensor_tensor(W4[:, 128:256], A0[:], BB1[:], op=Alu.mult)
                    nc.gpsimd.tensor_tensor(W4[:, 256:384], A1[:], BB0[:], op=Alu.mult)
                    nc.gpsimd.tensor_tensor(W4[:, 384:512], A1[:], BB1[:], op=Alu.mult)
                    W4s.append(W4)

                    # clamped f32 indices
                    yc0 = wk.tile([128, 128], dt.float32, tag="yc0")
                    nc.vector.tensor_scalar(yc0[:], Yf[:], 64.0, None, op0=Alu.max)
                    nc.vector.tensor_scalar(yc0[:], yc0[:], 191.0, None, op0=Alu.min)
                    yc1 = wk.tile([128, 128], dt.float32, tag="yc1")
                    nc.vector.tensor_scalar(yc1[:], Yf[:], 63.0, None, op0=Alu.max)
                    nc.vector.tensor_scalar(yc1[:], yc1[:], 190.0, None, op0=Alu.min)
                    xc = wk.tile([128, 128], dt.float32, tag="xc")
                    nc.vector.tensor_scalar(xc[:], Xf[:], 64.0, None, op0=Alu.max)
                    nc.vector.tensor_scalar(xc[:], xc[:], 191.0, None, op0=Alu.min)
                    idxT = wk.tile([128, 128], dt.float32, tag="idxT")
                    nc.vector.scalar_tensor_tensor(
                        idxT[:], yc0[:], 128.0, xc[:], op0=Alu.mult, op1=Alu.add
                    )
                    nc.vector.tensor_scalar(idxT[:], idxT[:], -8256.0, None, op0=Alu.add)
                    idxB = wk.tile([128, 128], dt.float32, tag="idxB")
                    nc.vector.scalar_tensor_tensor(
                        idxB[:], yc1[:], 128.0, xc[:], op0=Alu.mult, op1=Alu.add
                    )
                    nc.vector.tensor_scalar(idxB[:], idxB[:], -8128.0, None, op0=Alu.add)

                    # wrap to dma_gather layout via 8 permutation matmuls
                    IWt = wpool.tile([128, 1024], dt.int16, tag=f"IWt_{k}")
                    IWb = wpool.tile([128, 1024], dt.int16, tag=f"IWb_{k}")
                    for src, dstw in ((idxT, IWt), (idxB, IWb)):
                        for wg in range(8):
                            pw = psB.tile([128, 128], dt.float32, space="PSUM", tag="ps")
                            nc.tensor.matmul(pw[:], Rm[wg][:], src[:], start=True, stop=True)
                            dw = dstw[:]
                            nc.scalar.activation(
                                bass.AP(dw.tensor, dw.offset + wg,
                                        [dw.ap[0], [128, 8], [8, 16]]),
                                pw[:],
                                ACT.Copy,
                            )
                    IWts.append(IWt)
                    IWbs.append(IWb)

            # ---- phase 4: gather, weight, transpose, store --------------
            in_view = bass.AP(nhwc, 0, [[C, HW], [1, 2 * C]])
            with (
                tc.tile_pool(name="g", bufs=3) as gp,
                tc.tile_pool(name="t", bufs=3) as tp,
                tc.tile_pool(name="ev", bufs=8) as evp,
            ):
                for k in range(K):
                    W4 = W4s[k][:]
                    for blk in range(NBLK):
                        gt = gp.tile([128, J, 2 * C], dt.float32, tag="gt")
                        gb = gp.tile([128, J, 2 * C], dt.float32, tag="gb")
                        nc.gpsimd.dma_gather(
                            out_ap=gt[:], in_ap=in_view,
                            idxs_ap=IWts[k][:, blk * 128 : (blk + 1) * 128],
                            num_idxs=J * 128, num_idxs_reg=J * 128,
                            elem_size=2 * C, elem_step=C, single_packet=False,
                        )
                        nc.gpsimd.dma_gather(
                            out_ap=gb[:], in_ap=in_view,
                            idxs_ap=IWbs[k][:, blk * 128 : (blk + 1) * 128],
                            num_idxs=J * 128, num_idxs_reg=J * 128,
                            elem_size=2 * C, elem_step=C, single_packet=False,
                            queue_num=1,
                        )
                        tt = tp.tile([128, 2, J, C], dt.float32, tag="tt")
                        tb = tp.tile([128, 2, J, C], dt.float32, tag="tb")
                        wtop = bass.AP(
                            W4.tensor, W4.offset + blk * J,
                            [W4.ap[0], [1, J], [128, 2], [0, C]],
                        )
                        wbot = bass.AP(
                            W4.tensor, W4.offset + 256 + blk * J,
                            [W4.ap[0], [1, J], [128, 2], [0, C]],
                        )
                        gt_v = gt[:].rearrange("p j (s c) -> p j s c", s=2)
                        gb_v = gb[:].rearrange("p j (s c) -> p j s c", s=2)
                        tt_a = tt[:]
                        tb_a = tb[:]
                        tt_v = bass.AP(tt_a.tensor, tt_a.offset,
                                       [tt_a.ap[0], [C, J], [J * C, 2], [1, C]])
                        tb_v = bass.AP(tb_a.tensor, tb_a.offset,
                                       [tb_a.ap[0], [C, J], [J * C, 2], [1, C]])
                        nc.vector.tensor_tensor(tt_v, gt_v, wtop, op=Alu.mult)
                        nc.vector.tensor_tensor(tb_v, gb_v, wbot, op=Alu.mult)

                        for jj in range(J // 2):
                            pt = psC.tile([128, 128], dt.float32, space="PSUM", tag="pt")
                            first = True
                            for tl in (tt, tb):
                                a = tl[:]
                                for sl in range(2):
                                    src_ap = bass.AP(
                                        a.tensor,
                                        a.offset + sl * J * C + jj * 2 * C,
                                        [a.ap[0], [1, 2 * C]],
                                    )
                                    nc.tensor.matmul(
                                        pt[:], src_ap, ident[:],
                                        is_transpose=True,
                                        start=first, stop=(tl is tb and sl == 1),
                                    )
                                    first = False
                            ev = evp.tile([128, 128], dt.float32, tag="ev")
                            nc.scalar.activation(ev[:], pt[:], ACT.Copy)
                            nc.sync.dma_start(
                                out=bass.AP(
                                    col,
                                    k * HW + (blk * J + 2 * jj) * 128,
                                    [[128, 2], [K * HW, C], [1, 128]],
                                ),
                                in_=ev[:],
                            )

            psAB_cm.__exit__(None, None, None)

    nc.compile()
    return nc


_NC = None


def kernel(data_im, offset, mask):
    global _NC
    if _NC is None:
        _NC = _build()
    N = data_im.shape[0]
    in_maps = []
    for n in range(N):
        in_maps.append(
            dict(
                x_im=np.ascontiguousarray(data_im[n].reshape(C, HW), np.float32),
                x_off=np.ascontiguousarray(offset[n].reshape(18, HW), np.float32),
                x_mask=np.ascontiguousarray(mask[n].reshape(K, HW), np.float32),
            )
        )
    res = run_bass_kernel_spmd(_NC, in_maps, core_ids=list(range(N)))
    out = np.empty((C * K, N, H, W), np.float32)
    for n in range(N):
        out[:, n] = res.results[n]["col"].reshape(C * K, H, W)
    return out



# revision 3
# speedup vs baseline: 1.3080x; 1.3080x over previous
"""Deformable-conv im2col kernel for Trainium2 (8 NeuronCores, batch-parallel).

Contract: kernel(**inputs) takes the FULL inputs (data_im [8,64,128,128],
offset [8,18,128,128], mask [8,9,128,128]) and returns col [576,8,128,128].
Each of the 8 cores processes one batch image (SPMD, no collectives).

Per-core algorithm:
  1. Transpose the image NCHW -> NHWC into a DRAM scratch (PE transposes).
  2. Compute per-(tap k, output pixel) bilinear slot-weights and gather
     record indices on [wo-partition, ho-free] tiles (DVE/ACT).
  3. Rearrange indices into dma_gather's wrapped int16 layout with 8
     constant permutation matmuls on PE.
  4. Per (k, 16-row block): two dma_gathers fetch 512B row-segments
     (both x-corners x 64 channels, 256B-granular overlapping records),
     one broadcast-multiply per gather applies the 4 slot weights, and
     4 PSUM-accumulated PE transposes sum the corners while transposing
     pixels-on-partitions -> channels-on-partitions for the output DMA.

Host path: one cached jitted shard_map executable; inputs are passed as
zero-copy reshaped views sharded over batch; the col output is sharded
over axis 1 so the returned [576, 8*16384] buffer reshapes to
[576, 8, 128, 128] without a host transpose.
"""

import sys

sys.path.insert(0, "/opt/trn_rl_repo")

import numpy as np

import concourse.bass as bass
import concourse.bacc as bacc
import concourse.mybir as mybir
import concourse.tile as tile
from concourse.masks import make_identity

dt = mybir.dt
Alu = mybir.AluOpType
ACT = mybir.ActivationFunctionType

H = W = 128
C = 64
K = 9
HW = H * W  # 16384
J = 16      # output rows per gather round
NBLK = H // J  # 8
NREC = HW + 1  # gather records incl. one pad record


def _build():
    nc = bacc.Bacc("TRN2", target_bir_lowering=False, debug=False,
                   dynamic_dma_scratch_size=32768, num_swdge_queues=2)

    x_im = nc.dram_tensor("x_im", [C, HW], dt.float32, kind="ExternalInput")
    x_off = nc.dram_tensor("x_off", [18, HW], dt.float32, kind="ExternalInput")
    x_mask = nc.dram_tensor("x_mask", [K, HW], dt.float32, kind="ExternalInput")
    col = nc.dram_tensor("col", [C * K, HW], dt.float32, kind="ExternalOutput")
    nhwc = nc.dram_tensor("nhwc", [NREC * C], dt.float32, kind="Internal")

    with tile.TileContext(nc) as tc:
        # ---- persistent pools -------------------------------------------
        with (
            tc.tile_pool(name="const", bufs=1) as cpool,
            tc.tile_pool(name="wts", bufs=1) as wpool,
        ):
            ident = cpool.tile([128, 128], dt.float32)
            make_identity(nc, ident[:])

            psAB_cm = tc.tile_pool(name="psAB", bufs=4, space="PSUM")
            psA = psB = psAB_cm.__enter__()
            psC = psA

            # ---- phase 0: NCHW -> NHWC in DRAM --------------------------
            with tc.tile_pool(name="ph0", bufs=1) as p0, tc.tile_pool(
                name="ph0s", bufs=4
            ) as p0s:
                im_sb = p0.tile([C, HW], dt.float32)
                nc.sync.dma_start(out=im_sb[:], in_=x_im[:])
                zrow = p0s.tile([1, C], dt.float32)
                nc.gpsimd.memset(zrow[:], 0.0)
                nc.sync.dma_start(
                    out=bass.AP(nhwc, HW * C, [[1, C]]), in_=zrow[:]
                )
                for b in range(H):
                    ps = psA.tile([128, C], dt.float32, space="PSUM")
                    nc.tensor.transpose(
                        ps[:], im_sb[:, b * 128 : (b + 1) * 128], ident[0:C, 0:C]
                    )
                    st = p0s.tile([128, C], dt.float32)
                    nc.scalar.activation(st[:], ps[:], ACT.Copy)
                    nc.sync.dma_start(
                        out=bass.AP(nhwc, b * 128 * C, [[C, 128], [1, C]]),
                        in_=st[:],
                    )

            # ---- phase 1: transpose offsets+mask to [wo, q*128+ho] ------
            OT = wpool.tile([128, 27 * 128], dt.float32)
            with tc.tile_pool(name="ph1", bufs=1) as p1:
                om = p1.tile([27, HW], dt.float32)
                nc.sync.dma_start(out=om[0:18, :], in_=x_off[:])
                nc.sync.dma_start(out=om[18:27, :], in_=x_mask[:])
                for b in range(H):
                    ps = psA.tile([128, 27], dt.float32, space="PSUM")
                    nc.tensor.transpose(
                        ps[:], om[:, b * 128 : (b + 1) * 128], ident[0:27, 0:27]
                    )
                    ot = OT[:]
                    nc.scalar.activation(
                        bass.AP(ot.tensor, ot.offset + b, [ot.ap[0], [128, 27]]),
                        ps[:],
                        ACT.Copy,
                    )

            # ---- phase 2: constants -------------------------------------
            iota_i = cpool.tile([128, 128], dt.int32)
            nc.gpsimd.iota(iota_i[:], pattern=[[1, 128]], base=0, channel_multiplier=0)
            iota_ho = cpool.tile([128, 128], dt.float32)
            nc.vector.tensor_copy(iota_ho[:], iota_i[:])
            iwo_i = cpool.tile([128, 1], dt.int32)
            nc.gpsimd.iota(iwo_i[:], pattern=[[1, 1]], base=0, channel_multiplier=1)
            iwo_f = cpool.tile([128, 1], dt.float32)
            nc.vector.tensor_copy(iwo_f[:], iwo_i[:])
            kxb = cpool.tile([128, K], dt.float32)
            for k in range(K):
                kx = k % 3
                nc.vector.tensor_scalar(
                    kxb[:, k : k + 1], iwo_f[:], float(kx - 1 + 63.5), None, op0=Alu.add
                )
            ones = cpool.tile([128, 128], dt.float32)
            nc.gpsimd.memset(ones[:], 1.0)
            # R matrices: R[wg][q, p] = 1 iff q == wg*16 + p%16
            Rm = []
            for wg in range(8):
                r = cpool.tile([128, 128], dt.float32, tag=f"R{wg}")
                nc.gpsimd.affine_select(
                    r[:],
                    ones[:],
                    pattern=[[0, 8], [1, 16]],
                    base=wg * 16,
                    channel_multiplier=-1,
                    compare_op=Alu.is_equal,
                    fill=0.0,
                )
                Rm.append(r)

            # ---- phase 3: weights + wrapped indices per tap -------------
            W4s, IWts, IWbs = [], [], []
            with tc.tile_pool(name="wk", bufs=2) as wk:
                for k in range(K):
                    ky = k // 3
                    OY = OT[:, (2 * k) * 128 : (2 * k + 1) * 128]
                    OX = OT[:, (2 * k + 1) * 128 : (2 * k + 2) * 128]
                    M = OT[:, (18 + k) * 128 : (19 + k) * 128]

                    pyA = wk.tile([128, 128], dt.float32, tag="pyA")
                    nc.vector.scalar_tensor_tensor(
                        pyA[:], OY, float(ky - 1 + 63.5), iota_ho[:],
                        op0=Alu.add, op1=Alu.add,
                    )
                    pxA = wk.tile([128, 128], dt.float32, tag="pxA")
                    nc.vector.tensor_scalar(pxA[:], OX, kxb[:, k : k + 1], None, op0=Alu.add)

                    Yi = wk.tile([128, 128], dt.int32, tag="Yi")
                    nc.vector.tensor_copy(Yi[:], pyA[:])
                    Xi = wk.tile([128, 128], dt.int32, tag="Xi")
                    nc.vector.tensor_copy(Xi[:], pxA[:])
                    Yf = wk.tile([128, 128], dt.float32, tag="Yf")
                    nc.vector.tensor_copy(Yf[:], Yi[:])
                    Xf = wk.tile([128, 128], dt.float32, tag="Xf")
                    nc.vector.tensor_copy(Xf[:], Xi[:])

                    ly = wk.tile([128, 128], dt.float32, tag="ly")
                    nc.vector.scalar_tensor_tensor(
                        ly[:], pyA[:], 0.5, Yf[:], op0=Alu.add, op1=Alu.subtract
                    )
                    lx = wk.tile([128, 128], dt.float32, tag="lx")
                    nc.vector.scalar_tensor_tensor(
                        lx[:], pxA[:], 0.5, Xf[:], op0=Alu.add, op1=Alu.subtract
                    )
                    omly = wk.tile([128, 128], dt.float32, tag="omly")
                    nc.scalar.activation(omly[:], ly[:], ACT.Copy, bias=1.0, scale=-1.0)
                    omlx = wk.tile([128, 128], dt.float32, tag="omlx")
                    nc.scalar.activation(omlx[:], lx[:], ACT.Copy, bias=1.0, scale=-1.0)

                    def rng_mask(src, lo, hi, tag):
                        a = wk.tile([128, 128], dt.float32, tag=tag + "a")
                        nc.vector.tensor_scalar(a[:], src[:], lo, None, op0=Alu.is_ge)
                        b2 = wk.tile([128, 128], dt.float32, tag=tag + "b")
                        nc.vector.tensor_scalar(b2[:], src[:], hi, None, op0=Alu.is_le)
                        o = wk.tile([128, 128], dt.float32, tag=tag + "o")
                        nc.gpsimd.tensor_tensor(o[:], a[:], b2[:], op=Alu.mult)
                        return o

                    vy0 = rng_mask(Yi, 64, 191, "vy0")
                    vy1 = rng_mask(Yi, 63, 190, "vy1")
                    vx0 = rng_mask(Xi, 64, 191, "vx0")
                    vx1 = rng_mask(Xi, 63, 190, "vx1")
                    e = wk.tile([128, 128], dt.float32, tag="e")
                    nc.vector.tensor_scalar(e[:], Xi[:], 63, None, op0=Alu.is_equal)
                    ge0 = wk.tile([128, 128], dt.float32, tag="ge0")
                    nc.vector.tensor_scalar(ge0[:], Xi[:], 64, None, op0=Alu.is_ge)

                    A0 = wk.tile([128, 128], dt.float32, tag="A0")
                    nc.vector.tensor_tensor(A0[:], omly[:], vy0[:], op=Alu.mult)
                    nc.vector.tensor_tensor(A0[:], A0[:], M, op=Alu.mult)
                    A1 = wk.tile([128, 128], dt.float32, tag="A1")
                    nc.vector.tensor_tensor(A1[:], ly[:], vy1[:], op=Alu.mult)
                    nc.vector.tensor_tensor(A1[:], A1[:], M, op=Alu.mult)
                    B0 = wk.tile([128, 128], dt.float32, tag="B0")
                    nc.vector.tensor_tensor(B0[:], omlx[:], vx0[:], op=Alu.mult)
                    B1 = wk.tile([128, 128], dt.float32, tag="B1")
                    nc.vector.tensor_tensor(B1[:], lx[:], vx1[:], op=Alu.mult)
                    BB0 = wk.tile([128, 128], dt.float32, tag="BB0")
                    nc.vector.tensor_tensor(BB0[:], e[:], B1[:], op=Alu.mult)
                    nc.vector.tensor_tensor(BB0[:], BB0[:], B0[:], op=Alu.add)
                    BB1 = wk.tile([128, 128], dt.float32, tag="BB1")
                    nc.vector.tensor_tensor(BB1[:], B1[:], ge0[:], op=Alu.mult)

                    W4 = wpool.tile([128, 512], dt.float32, tag=f"W4_{k}")
                    nc.gpsimd.tensor_tensor(W4[:, 0:128], A0[:], BB0[:], op=Alu.mult)
                    nc.gpsimd.tensor_tensor(W4[:, 128:256], A0[:], BB1[:], op=Alu.mult)
                    nc.gpsimd.tensor_tensor(W4[:, 256:384], A1[:], BB0[:], op=Alu.mult)
                    nc.gpsimd.tensor_tensor(W4[:, 384:512], A1[:], BB1[:], op=Alu.mult)
                    W4s.append(W4)

                    # clamped f32 indices
                    yc0 = wk.tile([128, 128], dt.float32, tag="yc0")
                    nc.vector.tensor_scalar(yc0[:], Yf[:], 64.0, None, op0=Alu.max)
                    nc.vector.tensor_scalar(yc0[:], yc0[:], 191.0, None, op0=Alu.min)
                    yc1 = wk.tile([128, 128], dt.float32, tag="yc1")
                    nc.vector.tensor_scalar(yc1[:], Yf[:], 63.0, None, op0=Alu.max)
                    nc.vector.tensor_scalar(yc1[:], yc1[:], 190.0, None, op0=Alu.min)
                    xc = wk.tile([128, 128], dt.float32, tag="xc")
                    nc.vector.tensor_scalar(xc[:], Xf[:], 64.0, None, op0=Alu.max)
                    nc.vector.tensor_scalar(xc[:], xc[:], 191.0, None, op0=Alu.min)
                    idxT = wk.tile([128, 128], dt.float32, tag="idxT")
                    nc.vector.scalar_tensor_tensor(
                        idxT[:], yc0[:], 128.0, xc[:], op0=Alu.mult, op1=Alu.add
                    )
                    nc.vector.tensor_scalar(idxT[:], idxT[:], -8256.0, None, op0=Alu.add)
                    idxB = wk.tile([128, 128], dt.float32, tag="idxB")
                    nc.vector.scalar_tensor_tensor(
                        idxB[:], yc1[:], 128.0, xc[:], op0=Alu.mult, op1=Alu.add
                    )
                    nc.vector.tensor_scalar(idxB[:], idxB[:], -8128.0, None, op0=Alu.add)

                    # wrap to dma_gather layout via 8 permutation matmuls
                    IWt = wpool.tile([128, 1024], dt.int16, tag=f"IWt_{k}")
                    IWb = wpool.tile([128, 1024], dt.int16, tag=f"IWb_{k}")
                    for src, dstw in ((idxT, IWt), (idxB, IWb)):
                        for wg in range(8):
                            pw = psB.tile([128, 128], dt.float32, space="PSUM", tag="ps")
                            nc.tensor.matmul(pw[:], Rm[wg][:], src[:], start=True, stop=True)
                            dw = dstw[:]
                            nc.scalar.activation(
                                bass.AP(dw.tensor, dw.offset + wg,
                                        [dw.ap[0], [128, 8], [8, 16]]),
                                pw[:],
                                ACT.Copy,
                            )
                    IWts.append(IWt)
                    IWbs.append(IWb)

            # ---- phase 4: gather, weight, transpose, store --------------
            in_view = bass.AP(nhwc, 0, [[C, HW], [1, 2 * C]])
            with (
                tc.tile_pool(name="g", bufs=3) as gp,
                tc.tile_pool(name="t", bufs=3) as tp,
                tc.tile_pool(name="ev", bufs=8) as evp,
            ):
                for k in range(K):
                    W4 = W4s[k][:]
                    for blk in range(NBLK):
                        gt = gp.tile([128, J, 2 * C], dt.float32, tag="gt")
                        gb = gp.tile([128, J, 2 * C], dt.float32, tag="gb")
                        nc.gpsimd.dma_gather(
                            out_ap=gt[:], in_ap=in_view,
                            idxs_ap=IWts[k][:, blk * 128 : (blk + 1) * 128],
                            num_idxs=J * 128, num_idxs_reg=J * 128,
                            elem_size=2 * C, elem_step=C, single_packet=False,
                        )
                        nc.gpsimd.dma_gather(
                            out_ap=gb[:], in_ap=in_view,
                            idxs_ap=IWbs[k][:, blk * 128 : (blk + 1) * 128],
                            num_idxs=J * 128, num_idxs_reg=J * 128,
                            elem_size=2 * C, elem_step=C, single_packet=False,
                            queue_num=1,
                        )
                        tt = tp.tile([128, 2, J, C], dt.float32, tag="tt")
                        tb = tp.tile([128, 2, J, C], dt.float32, tag="tb")
                        wtop = bass.AP(
                            W4.tensor, W4.offset + blk * J,
                            [W4.ap[0], [1, J], [128, 2], [0, C]],
                        )
                        wbot = bass.AP(
                            W4.tensor, W4.offset + 256 + blk * J,
                            [W4.ap[0], [1, J], [128, 2], [0, C]],
                        )
                        gt_v = gt[:].rearrange("p j (s c) -> p j s c", s=2)
                        gb_v = gb[:].rearrange("p j (s c) -> p j s c", s=2)
                        tt_a = tt[:]
                        tb_a = tb[:]
                        tt_v = bass.AP(tt_a.tensor, tt_a.offset,
                                       [tt_a.ap[0], [C, J], [J * C, 2], [1, C]])
                        tb_v = bass.AP(tb_a.tensor, tb_a.offset,
                                       [tb_a.ap[0], [C, J], [J * C, 2], [1, C]])
                        nc.vector.tensor_tensor(tt_v, gt_v, wtop, op=Alu.mult)
                        nc.vector.tensor_tensor(tb_v, gb_v, wbot, op=Alu.mult)

                        for jj in range(J // 2):
                            pt = psC.tile([128, 128], dt.float32, space="PSUM", tag="pt")
                            first = True
                            for tl in (tt, tb):
                                a = tl[:]
                                for sl in range(2):
                                    src_ap = bass.AP(
                                        a.tensor,
                                        a.offset + sl * J * C + jj * 2 * C,
                                        [a.ap[0], [1, 2 * C]],
                                    )
                                    nc.tensor.matmul(
                                        pt[:], src_ap, ident[:],
                                        is_transpose=True,
                                        start=first, stop=(tl is tb and sl == 1),
                                    )
                                    first = False
                            ev = evp.tile([128, 128], dt.float32, tag="ev")
                            nc.scalar.activation(ev[:], pt[:], ACT.Copy)
                            nc.sync.dma_start(
                                out=bass.AP(
                                    col,
                                    k * HW + (blk * J + 2 * jj) * 128,
                                    [[128, 2], [K * HW, C], [1, 128]],
                                ),
                                in_=ev[:],
                            )

            psAB_cm.__exit__(None, None, None)

    nc.compile()
    return nc


_NC = None
_EXEC = None


def _setup():
    """Build the NEFF once and cache a jitted sharded executable."""
    global _NC, _EXEC
    if _EXEC is not None:
        return _EXEC
    import jax
    from jax.sharding import Mesh, PartitionSpec
    import warnings

    with warnings.catch_warnings():
        warnings.simplefilter("ignore")
        from jax.experimental.shard_map import shard_map
    from concourse import bass2jax
    from concourse.bass2jax import (
        _bass_exec_p,
        install_neuronx_cc_hook,
        partition_id_tensor,
    )

    install_neuronx_cc_hook()
    if _NC is None:
        _NC = _build()
    nc = _NC

    in_names, out_names, out_avals, zero_outs = [], [], [], []
    for alloc in nc.m.functions[0].allocations:
        if not isinstance(alloc, mybir.MemoryLocationSet):
            continue
        name = alloc.memorylocations[0].name
        if alloc.kind == "ExternalInput":
            if nc.partition_id_tensor is None or name != nc.partition_id_tensor.name:
                in_names.append(name)
        elif alloc.kind == "ExternalOutput":
            shp = tuple(alloc.tensor_shape)
            dtp = mybir.dt.np(alloc.dtype)
            out_avals.append(jax.core.ShapedArray(shp, dtp))
            out_names.append(name)
            zero_outs.append(np.zeros(shp, dtp))
    n_params = len(in_names)
    all_in = tuple(in_names) + tuple(out_names)
    if nc.partition_id_tensor is not None:
        all_in = all_in + (nc.partition_id_tensor.name,)

    # x_im/x_off/x_mask order must match what kernel() passes
    assert in_names == ["x_im", "x_off", "x_mask"], in_names
    assert out_names == ["col"], out_names

    def _body(*args):
        args = list(args)
        if nc.partition_id_tensor is not None:
            args.append(partition_id_tensor())
        return tuple(
            _bass_exec_p.bind(
                *args,
                out_avals=tuple(out_avals),
                in_names=all_in,
                out_names=tuple(out_names),
                lowering_input_output_aliases=(),
                sim_require_finite=False,
                sim_require_nnan=False,
                nc=nc,
            )
        )

    n_dev = 8
    devices = jax.devices()[:n_dev]
    mesh = Mesh(np.array(devices), ("core",))
    # inputs sharded over batch (axis 0 blocks); col sharded over axis 1 so
    # the global result reshapes to [C*K, N, H, W] with zero host transpose
    in_specs = (PartitionSpec("core"),) * n_params + (PartitionSpec(None, "core"),)
    out_specs = (PartitionSpec(None, "core"),)
    sharded = jax.jit(
        shard_map(_body, mesh=mesh, in_specs=in_specs, out_specs=out_specs,
                  check_rep=False),
        keep_unused=True,
    )
    zeros_dev = jax.device_put(
        np.zeros((C * K, n_dev * HW), np.float32),
        jax.sharding.NamedSharding(mesh, PartitionSpec(None, "core")),
    )
    _EXEC = (sharded, zeros_dev)
    return _EXEC


def kernel(data_im, offset, mask):
    sharded, zeros_dev = _setup()
    N = data_im.shape[0]
    assert N == 8
    xim = np.ascontiguousarray(data_im, np.float32).reshape(N * C, HW)
    xoff = np.ascontiguousarray(offset, np.float32).reshape(N * 18, HW)
    xmask = np.ascontiguousarray(mask, np.float32).reshape(N * K, HW)
    (out,) = sharded(xim, xoff, xmask, zeros_dev)
    res = np.asarray(out)  # [C*K, N*HW] laid out as (ck, n, hw)
    return res.reshape(C * K, N, H, W)
